# revision 26
# baseline (speedup 1.0000x reference)
"""Trainium2 Bass kernel for nn_BDH_4406636445711 (dense transformer).

Sharding: 8 cores = data-parallel over B(2) x tensor-parallel over H(4).
Core c handles (b = c//4, h = c%4): its head's Dx/Dy slices, E rows, and a
V/4 shard of the readout. Per layer the y@E partial is AllReduced (bf16)
within each b-group of 4 cores. The host stitches the 8 per-core [VS, T]
logit shards (bf16 on device, cast to fp32 host-side) into [B, T, V].

Key algebraic optimization vs the naive graph: scores = q @ q^T is only
ever used for a = scores @ v, so we compute a = q @ (q^T v) instead --
G = q^T v is [K, D]; ~5x fewer PE cycles than materializing [T, T] scores.

Layouts: v lives token-major ("td": [128 tok part, 16 chunk, 256 d]) so
every LayerNorm is a free-dim reduction. x/q live kT; q is additionally
transposed to tk (via the DMA crossbar, not the PE) for the G matmul.
All matmul operands are bf16 (full PE rate); accumulation and LN stats
stay fp32.

Scheduling structure (v3):
- Each token half runs attention->Dy->Du back to back, so its AllReduce
  launches a half-phase earlier and rides under the other half's matmuls.
- LN statistics use one-pass bn_stats/bn_aggr; apply + residual-add +
  the vp transpose are chunk-pipelined across ACT/DVE/PE so the
  post-collective chain is short and px can start after 4 chunks.
- q's kT->tk transpose rides the DMA crossbar via a DRAM round-trip,
  with dispatches split across the SP and ACT hwdge queues.
- The readout streams weights in 512-column chunks on a 4-deep
  [128,512] PSUM pool, alternating DVE/ACT for the PSUM->SBUF casts.
"""

import os
import sys

sys.path.insert(0, "/opt/trn_rl_repo")

import numpy as np

import concourse.bass as bass
import concourse.tile as tile
from concourse import bacc, mybir
from concourse.bass_utils import run_bass_kernel_spmd
from concourse.masks import make_identity
from concourse import library_config

F32 = mybir.dt.float32
BF16 = mybir.dt.bfloat16
I32 = mybir.dt.int32
AF = mybir.ActivationFunctionType
OP = mybir.AluOpType
AX = mybir.AxisListType

B, T, H, D, K, V, L = 2, 2048, 4, 256, 1024, 32000, 6
VS = V // 4          # vocab shard per core within a b-group
EPS = 1e-5
NT = T // 128        # 16 token chunks
NKC = K // 128       # 8 k chunks
ND = D // 128        # 2 d chunks
TH = T // 2          # 1024

N_LAYERS = int(os.environ.get("KRN_LAYERS", str(L)))
DO_READOUT = os.environ.get("KRN_READOUT", "1") == "1"


def build(nc):
    # ---- DRAM parameters (per core) ----
    tok_d = nc.dram_tensor("tok", [T], I32, kind="ExternalInput")
    emb_d = nc.dram_tensor("emb", [V, D], F32, kind="ExternalInput")
    pos_d = nc.dram_tensor("posb", [T, D], BF16, kind="ExternalInput")
    dx_d = nc.dram_tensor("dxb", [D, K], BF16, kind="ExternalInput")
    dy_d = nc.dram_tensor("dyb", [D, K], BF16, kind="ExternalInput")
    e_d = nc.dram_tensor("eb", [K, D], BF16, kind="ExternalInput")
    ro_d = nc.dram_tensor("rob", [D, VS], BF16, kind="ExternalInput")
    cos_d = nc.dram_tensor("cosb", [8, 128, TH], BF16, kind="ExternalInput")
    sin_d = nc.dram_tensor("sinb", [8, 128, TH], BF16, kind="ExternalInput")
    out_d = nc.dram_tensor("logitsT", [VS, T], BF16, kind="ExternalOutput")

    groups = [[0, 1, 2, 3], [4, 5, 6, 7]]

    with tile.TileContext(nc) as tc:
        with (
            nc.allow_low_precision(reason="bf16 matmul path is intentional"),
            tc.tile_pool(name="persist", bufs=1) as pp,
            tc.tile_pool(name="w4", bufs=4) as w4p,     # [128,1024] bf16 rope
            tc.tile_pool(name="stats", bufs=24) as stp, # [128,8] f32
            tc.tile_pool(name="bst", bufs=12) as bsp,   # [128,8,6]/[128,8,2] f32
            tc.tile_pool(name="rop", bufs=2) as rop,    # readout weights
            tc.tile_pool(name="lop", bufs=2) as lop,    # logit staging
            tc.tile_pool(name="pb", bufs=4, space="PSUM") as pbp,   # [128,512] f32
            tc.tile_pool(name="pa", bufs=2, space="PSUM") as pap,   # [128,256] f32
            tc.tile_pool(name="pt", bufs=2, space="PSUM") as ptp,   # [128,1024] bf16
            tc.tile_pool(name="dram", bufs=1, space="DRAM") as dpool,
        ):
            _ctr = [0]

            def _nm(p):
                _ctr[0] += 1
                return f"{p}{_ctr[0]}"

            # ---- constants ----
            ident_f = pp.tile([128, 128], F32)
            make_identity(nc, ident_f[:])
            ident_bf = pp.tile([128, 128], BF16)
            nc.vector.tensor_copy(ident_bf[:], ident_f[:])
            eps_p = pp.tile([128, 1], F32)
            nc.vector.memset(eps_p[:], EPS)
            nc.gpsimd.load_library(library_config.attn)

            # ---- persistent tensors ----
            v_td = pp.tile([128, NT, D], F32)          # ln(w), token-major (no pos)
            vp_bf = pp.tile([128, NT, D], BF16)        # v + pos, bf16
            vpT = pp.tile([128, ND, T], BF16)          # (v+pos) transposed
            qT = pp.tile([128, NKC, T], BF16)          # q k-major; reused as yt
            qtk = pp.tile([128, NT, K], BF16)          # q token-major
            x_bf = pp.tile([128, NKC, T], BF16)        # relu(v@Dx), k-major
            g0_sb = pp.tile([128, NKC, D], BF16)       # G partial (th0 tokens)
            g_bf = pp.tile([128, NKC, D], BF16)        # G = q^T (v+pos), full
            lnA_dT = pp.tile([128, ND, T], BF16)       # ln(a) d-major
            pos_sb = pp.tile([128, NT, D], BF16)
            dx_sb = pp.tile([128, ND, K], BF16)
            dy_sb = pp.tile([128, ND, K], BF16)
            e_sb = pp.tile([128, NKC, D], BF16)
            # half-batch scratch ([128, 8, 256] = one token half). Multi-purpose;
            # phase ordering keeps uses disjoint (WAR tracked by the framework).
            a0_all = pp.tile([128, 8, D], BF16)        # th0: a / u-stage / u
            a1_all = pp.tile([128, 8, D], BF16)        # th1: a / u-stage / u
            uln_all = pp.tile([128, 8, D], F32)        # ln(u) / embed gathers

            nc.sync.dma_start(pos_sb[:], pos_d.ap().rearrange("(j p) d -> p j d", p=128))
            nc.sync.dma_start(dx_sb[:], dx_d.ap().rearrange("(c p) k -> p c k", p=128))
            nc.sync.dma_start(dy_sb[:], dy_d.ap().rearrange("(c p) k -> p c k", p=128))
            nc.sync.dma_start(e_sb[:], e_d.ap().rearrange("(c p) d -> p c d", p=128))

            # ---- internal DRAM ----
            cc_in = [dpool.tile([8, 128, D], BF16, tag=f"cci{i}", name=f"cci{i}")
                     for i in range(2)]
            cc_out = [dpool.tile([8, 128, D], BF16, tag=f"cco{i}", name=f"cco{i}")
                      for i in range(2)]
            q_dr = dpool.tile([K, T], BF16, tag="qdr", name="qdr")

            def ln_tail(mv):
                """(mean, var) [128,8,2] -> (rstd, nmr) [128,8]."""
                sd = stp.tile([128, 8], F32, tag="st", name=_nm("st_"))
                nc.scalar.activation(sd[:], mv[:, :, 1], AF.Sqrt, bias=eps_p[:])
                rstd = stp.tile([128, 8], F32, tag="st", name=_nm("st_"))
                nc.vector.reciprocal(rstd[:], sd[:])
                nmr = stp.tile([128, 8], F32, tag="st", name=_nm("st_"))
                nc.vector.scalar_tensor_tensor(
                    out=nmr[:], in0=mv[:, :, 0], scalar=-1.0, in1=rstd[:],
                    op0=OP.mult, op1=OP.mult)
                return rstd, nmr

            def ln_stats(src3d):
                """One-pass LN stats of [128, 8, 256] -> (rstd, nmr) [128,8]."""
                bst = bsp.tile([128, 8, 6], F32, tag="bst6", name=_nm("bs_"))
                for j in range(8):
                    nc.vector.bn_stats(bst[:, j, :], src3d[:, j, :])
                mv = bsp.tile([128, 8, 2], F32, tag="bst2", name=_nm("bs_"))
                for j in range(8):
                    nc.vector.bn_aggr(mv[:, j, :], bst[:, j, :])
                return ln_tail(mv)

            def apply_half(dst_tile, dst0, src_tile, src0, rstd, nmr):
                for j8 in range(8):
                    nc.scalar.activation(dst_tile[:, dst0 + j8],
                                         src_tile[:, src0 + j8], AF.Identity,
                                         bias=nmr[:, j8:j8 + 1],
                                         scale=rstd[:, j8:j8 + 1])

            def transpose_half(src_tile, sl, dst, c0):
                """Transpose 8 [128, 256] td chunks into dst[:, dc, c0:c0+1024]."""
                tpa = ptp.tile([128, TH], BF16, tag="pt", name=_nm("pt_"))
                tpb = ptp.tile([128, TH], BF16, tag="pt", name=_nm("pt_"))
                for j8 in range(8):
                    nc.tensor.transpose(tpa[:, j8 * 128:(j8 + 1) * 128],
                                        src_tile[:, sl + j8, 0:128], ident_bf[:])
                    nc.tensor.transpose(tpb[:, j8 * 128:(j8 + 1) * 128],
                                        src_tile[:, sl + j8, 128:256], ident_bf[:])
                nc.scalar.copy(dst[:, 0, c0:c0 + TH], tpa[:])
                nc.scalar.copy(dst[:, 1, c0:c0 + TH], tpb[:])

            def phaseB_px(th):
                """x[:, th cols] = relu(vp @ Dx)."""
                c0 = th * TH

                def px_one(i):
                    for ns in range(2):
                        px = pbp.tile([128, 512], F32, tag="pb", name=_nm("pb_"))
                        for dc in range(ND):
                            nc.tensor.matmul(
                                px[:],
                                dx_sb[:, dc, i * 128:(i + 1) * 128],
                                vpT[:, dc, c0 + ns * 512:c0 + (ns + 1) * 512],
                                start=(dc == 0), stop=(dc == ND - 1))
                        nc.scalar.activation(
                            x_bf[:, i, c0 + ns * 512:c0 + (ns + 1) * 512],
                            px[:], AF.Relu)

                for i in range(NKC):
                    px_one(i)

            def phaseB_rope(th):
                """RoPE -> q; q -> DRAM -> qtk (one batched xbar transpose)."""
                c0 = th * TH

                def rope_one(i):
                    """q_i on DVE, q_{i+4} mostly on Pool (parallel chains)."""
                    cos_t = w4p.tile([128, TH], BF16, tag="w4", name=_nm("w4_"))
                    nc.sync.dma_start(cos_t[:], cos_d.ap()[i * 2 + th])
                    sin_t = w4p.tile([128, TH], BF16, tag="w4", name=_nm("w4_"))
                    nc.sync.dma_start(sin_t[:], sin_d.ap()[i * 2 + th])
                    xi = x_bf[:, i, c0:c0 + TH]
                    xj = x_bf[:, i + 4, c0:c0 + TH]
                    ma = w4p.tile([128, TH], BF16, tag="w4", name=_nm("w4_"))
                    nc.vector.tensor_mul(ma[:], xi, cos_t[:])
                    mb = w4p.tile([128, TH], BF16, tag="w4", name=_nm("w4_"))
                    nc.vector.tensor_mul(mb[:], xj, sin_t[:])
                    nc.vector.tensor_sub(qT[:, i, c0:c0 + TH], ma[:], mb[:])
                    nc.vector.tensor_mul(ma[:], xj, cos_t[:])
                    nc.vector.tensor_mul(mb[:], xi, sin_t[:])
                    nc.vector.tensor_add(qT[:, i + 4, c0:c0 + TH], ma[:], mb[:])
                    nc.sync.dma_start(q_dr[i * 128:(i + 1) * 128, c0:c0 + TH],
                                      qT[:, i, c0:c0 + TH])
                    nc.sync.dma_start(
                        q_dr[(i + 4) * 128:(i + 5) * 128, c0:c0 + TH],
                        qT[:, i + 4, c0:c0 + TH])

                rope_one(0); rope_one(1); rope_one(2); rope_one(3)
                # NOTE: xbar transposes dispatched from the ACT hwdge queue
                # return corrupted data on HW -- keep them on SP. Two batched
                # dispatches (k-halves, matching the DVE/Pool chain split):
                # out[t, j, k] = q_dr[k, c0 + j*128 + t]. G's first chains can
                # start as soon as the kh0 half lands.
                for kh in range(2):
                    nc.sync.dma_start_transpose(
                        qtk[:, th * 8:th * 8 + 8, kh * 512:(kh + 1) * 512],
                        q_dr[kh * 512:(kh + 1) * 512, c0:c0 + TH])

            def phaseB(th):
                phaseB_px(th)
                phaseB_rope(th)

            def phaseG(half):
                """G half-accumulation over token chunks; half 1 finalizes g_bf."""
                for kc in range(NKC):
                    pg = pap.tile([128, D], F32, tag="pa", name=_nm("pa_"))
                    for j in range(half * 8, half * 8 + 8):
                        nc.tensor.matmul(pg[:], qtk[:, j, kc * 128:(kc + 1) * 128],
                                         vp_bf[:, j],
                                         start=(j == half * 8), stop=(j == half * 8 + 7))
                    if half == 0:
                        nc.scalar.copy(g0_sb[:, kc], pg[:])
                    else:
                        nc.vector.tensor_add(g_bf[:, kc], g0_sb[:, kc], pg[:])

            def phaseCa_mm(half, dst_tile):
                """a = q G matmuls for a token half; PSUM -> SBUF bf16."""
                h0 = half * 8
                for j8 in range(8):
                    j = h0 + j8
                    paa = pap.tile([128, D], F32, tag="pa", name=_nm("pa_"))
                    for kc in range(NKC):
                        nc.tensor.matmul(paa[:], qT[:, kc, j * 128:(j + 1) * 128],
                                         g_bf[:, kc],
                                         start=(kc == 0), stop=(kc == NKC - 1))
                    nc.scalar.copy(dst_tile[:, j8], paa[:])

            def phaseCa_fin(half, src_tile):
                """Batched LN of the a half (in place) -> lnA_dT, chunk-piped."""
                c0 = half * TH
                rstd, nmr = ln_stats(src_tile[:])
                tpa = ptp.tile([128, TH], BF16, tag="pt", name=_nm("pt_"))
                tpb = ptp.tile([128, TH], BF16, tag="pt", name=_nm("pt_"))
                for j in range(8):
                    nc.scalar.activation(src_tile[:, j], src_tile[:, j],
                                         AF.Identity, bias=nmr[:, j:j + 1],
                                         scale=rstd[:, j:j + 1])
                    nc.tensor.transpose(tpa[:, j * 128:(j + 1) * 128],
                                        src_tile[:, j, 0:128], ident_bf[:])
                    nc.tensor.transpose(tpb[:, j * 128:(j + 1) * 128],
                                        src_tile[:, j, 128:256], ident_bf[:])
                    if j == 3:
                        nc.scalar.copy(lnA_dT[:, 0, c0:c0 + 512], tpa[:, 0:512])
                        nc.scalar.copy(lnA_dT[:, 1, c0:c0 + 512], tpb[:, 0:512])
                nc.scalar.copy(lnA_dT[:, 0, c0 + 512:c0 + TH], tpa[:, 512:TH])
                nc.scalar.copy(lnA_dT[:, 1, c0 + 512:c0 + TH], tpb[:, 512:TH])

            def phaseDy(th):
                """y = relu(lnA@Dy)*x into yt (aliases q's buffer)."""
                c0 = th * TH
                yt = qT
                for i in range(NKC):
                    for ns in range(2):
                        py = pbp.tile([128, 512], F32, tag="pb", name=_nm("pb_"))
                        for dc in range(ND):
                            nc.tensor.matmul(
                                py[:],
                                dy_sb[:, dc, i * 128:(i + 1) * 128],
                                lnA_dT[:, dc, c0 + ns * 512:c0 + (ns + 1) * 512],
                                start=(dc == 0), stop=(dc == ND - 1))
                        nc.vector.scalar_tensor_tensor(
                            out=yt[:, i, c0 + ns * 512:c0 + (ns + 1) * 512],
                            in0=py[:], scalar=0.0,
                            in1=x_bf[:, i, c0 + ns * 512:c0 + (ns + 1) * 512],
                            op0=OP.max, op1=OP.mult)

            def phaseDu(th, stage):
                """u = y@E (token-major); stage bf16 and AllReduce."""
                yt = qT
                for j8 in range(8):
                    j = th * 8 + j8
                    pu = pap.tile([128, D], F32, tag="pa", name=_nm("pa_"))
                    for i in range(NKC):
                        nc.tensor.matmul(pu[:], yt[:, i, j * 128:(j + 1) * 128],
                                         e_sb[:, i],
                                         start=(i == 0), stop=(i == NKC - 1))
                    nc.scalar.copy(stage[:, j8], pu[:])
                    if j8 == 3:
                        nc.sync.dma_start(
                            cc_in[th][0:4].rearrange("j p d -> p j d"),
                            stage[:, 0:4])
                nc.sync.dma_start(
                    cc_in[th][4:8].rearrange("j p d -> p j d"),
                    stage[:, 4:8])
                nc.gpsimd.collective_compute(
                    "AllReduce", OP.add, replica_groups=groups,
                    ins=[cc_in[th][:].opt()], outs=[cc_out[th][:].opt()])

            def phaseE_posadd(th):
                sl = slice(th * 8, th * 8 + 8)
                nc.vector.tensor_add(v_td[:, sl], v_td[:, sl], pos_sb[:, sl])

            def phaseE_udma(th, ubuf):
                """Bring the AllReduced u back in two chunks."""
                for hh in range(2):
                    nc.sync.dma_start(
                        ubuf[:, hh * 4:(hh + 1) * 4],
                        cc_out[th][hh * 4:(hh + 1) * 4].rearrange("j p d -> p j d"))

            def phaseE_chain_mono(th, layer, ubuf):
                """v2-style monolithic E chain for bisection."""
                last = layer == N_LAYERS - 1
                h0 = th * 8
                sl = slice(h0, h0 + 8)
                rstd_u, nmr_u = ln_stats(ubuf[:])
                apply_half(uln_all, 0, ubuf, 0, rstd_u, nmr_u)
                nc.vector.tensor_add(v_td[:, sl], v_td[:, sl], uln_all[:])
                rstd_w, nmr_w = ln_stats(v_td[:, sl])
                apply_half(v_td, h0, v_td, h0, rstd_w, nmr_w)
                if not last:
                    nc.vector.tensor_add(vp_bf[:, sl], v_td[:, sl], pos_sb[:, sl])
                else:
                    nc.scalar.copy(vp_bf[:, sl], v_td[:, sl])
                transpose_half(vp_bf, h0, vpT, th * TH)

            def phaseE_chain_chunked(th, layer, ubuf):
                """w = (v+pos)+ln(u); v = ln(w); vp = v+pos'; vpT. Chunk-piped
                across ACT/DVE/PE so px can start after 4 chunks."""
                last = layer == N_LAYERS - 1
                h0 = th * 8
                c0 = th * TH
                rstd_u, nmr_u = ln_stats(ubuf[:])
                bstw = bsp.tile([128, 8, 6], F32, tag="bst6", name=_nm("bs_"))
                for j in range(8):
                    nc.scalar.activation(uln_all[:, j], ubuf[:, j], AF.Identity,
                                         bias=nmr_u[:, j:j + 1],
                                         scale=rstd_u[:, j:j + 1])
                    nc.vector.tensor_add(v_td[:, h0 + j], v_td[:, h0 + j],
                                         uln_all[:, j])
                    nc.vector.bn_stats(bstw[:, j, :], v_td[:, h0 + j])
                mvw = bsp.tile([128, 8, 2], F32, tag="bst2", name=_nm("bs_"))
                for j in range(8):
                    nc.vector.bn_aggr(mvw[:, j, :], bstw[:, j, :])
                rstd_w, nmr_w = ln_tail(mvw)
                tpa = ptp.tile([128, TH], BF16, tag="pt", name=_nm("pt_"))
                tpb = ptp.tile([128, TH], BF16, tag="pt", name=_nm("pt_"))
                for j in range(8):
                    nc.scalar.activation(v_td[:, h0 + j], v_td[:, h0 + j],
                                         AF.Identity, bias=nmr_w[:, j:j + 1],
                                         scale=rstd_w[:, j:j + 1])
                    if not last:
                        nc.vector.tensor_add(vp_bf[:, h0 + j], v_td[:, h0 + j],
                                             pos_sb[:, h0 + j])
                    else:
                        nc.scalar.copy(vp_bf[:, h0 + j], v_td[:, h0 + j])
                    nc.tensor.transpose(tpa[:, j * 128:(j + 1) * 128],
                                        vp_bf[:, h0 + j, 0:128], ident_bf[:])
                    nc.tensor.transpose(tpb[:, j * 128:(j + 1) * 128],
                                        vp_bf[:, h0 + j, 128:256], ident_bf[:])
                    if j == 3:
                        nc.scalar.copy(vpT[:, 0, c0:c0 + 512], tpa[:, 0:512])
                        nc.scalar.copy(vpT[:, 1, c0:c0 + 512], tpb[:, 0:512])
                nc.scalar.copy(vpT[:, 0, c0 + 512:c0 + TH], tpa[:, 512:TH])
                nc.scalar.copy(vpT[:, 1, c0 + 512:c0 + TH], tpb[:, 512:TH])

            phaseE_chain = (phaseE_chain_chunked
                            if os.environ.get("KRN_CHAIN", "1") == "1"
                            else phaseE_chain_mono)

            # vocab block structure for the readout: 63 blocks of 128 (last 64),
            # weight chunks of 4 blocks (512 cols), stores in vblock pairs.
            VBW = [128] * 62 + [64]
            VCHUNKS = []
            vb = 0
            while vb < 63:
                VCHUNKS.append(list(range(vb, min(vb + 4, 63))))
                vb += 4
            _cp = [0]

            def readout_half(th):
                """logitsT[:, th cols] = (v @ readout)^T for the token half."""
                for chunk in VCHUNKS:
                    off = chunk[0] * 128
                    w = sum(VBW[i] for i in chunk)
                    rot = rop.tile([128, ND, 512], BF16, tag="ro", name=_nm("ro_"))
                    for dc in range(ND):
                        nc.sync.dma_start(
                            rot[:, dc, :w],
                            ro_d.ap()[dc * 128:(dc + 1) * 128, off:off + w])
                    pairs = [chunk[i:i + 2] for i in range(0, len(chunk), 2)]
                    for pair in pairs:
                        lo = lop.tile([128, len(pair), TH], BF16, tag="lo",
                                      name=_nm("lo_"))
                        for mi, vbi in enumerate(pair):
                            m = VBW[vbi]
                            loc = vbi * 128 - off
                            for ns in range(2):
                                pl = pbp.tile([128, 512], F32, tag="pb",
                                              name=_nm("pb_"))
                                for dc in range(ND):
                                    nc.tensor.matmul(
                                        pl[:m],
                                        rot[:, dc, loc:loc + m],
                                        vpT[:, dc, th * TH + ns * 512:
                                            th * TH + (ns + 1) * 512],
                                        start=(dc == 0), stop=(dc == ND - 1))
                                _cp[0] += 1
                                dst = lo[:m, mi, ns * 512:(ns + 1) * 512]
                                if _cp[0] % 2 == 0:
                                    nc.vector.tensor_copy(dst, pl[:m])
                                else:
                                    nc.scalar.copy(dst, pl[:m])
                        r0 = pair[0] * 128
                        rows = sum(VBW[i] for i in pair)
                        if len(pair) == 2 and rows == 256:
                            nc.sync.dma_start(
                                out_d.ap()[r0:r0 + 256, th * TH:(th + 1) * TH]
                                .rearrange("(vb p) t -> p vb t", p=128),
                                lo[:])
                        else:
                            nc.sync.dma_start(
                                out_d.ap()[r0:r0 + rows, th * TH:(th + 1) * TH],
                                lo[:rows, 0])

            # ======================= prologue: gather + LN =======================
            idx = pp.tile([128, NT], I32)
            nc.sync.dma_start(idx[:], tok_d.ap().rearrange("(n p) -> p n", p=128))

            def embed_gather(th):
                for j8 in range(8):
                    nc.gpsimd.indirect_dma_start(
                        out=uln_all[:, j8], out_offset=None, in_=emb_d.ap(),
                        in_offset=bass.IndirectOffsetOnAxis(
                            ap=idx[:, th * 8 + j8:th * 8 + j8 + 1], axis=0),
                    )

            def embed_half(th):
                h0 = th * 8
                sl = slice(h0, h0 + 8)
                rstd, nmr = ln_stats(uln_all[:])
                apply_half(v_td, h0, uln_all, 0, rstd, nmr)
                nc.vector.tensor_add(vp_bf[:, sl], v_td[:, sl], pos_sb[:, sl])
                transpose_half(vp_bf, h0, vpT, th * TH)

            embed_gather(0)
            embed_half(0)
            embed_gather(1)   # WAR on uln_all: starts once half0's apply read it
            phaseB(0)
            embed_half(1)
            phaseB(1)
            phaseG(0)
            phaseG(1)

            # ================================ layers ================================
            for layer in range(N_LAYERS):
                last = layer == N_LAYERS - 1
                with nc.named_scope(f"L{layer}"):
                    phaseCa_mm(0, a0_all)
                    phaseE_posadd(0)
                    phaseCa_fin(0, a0_all)
                    phaseDy(0)
                    phaseDu(0, a0_all)      # cc0 in flight...
                    phaseCa_mm(1, a1_all)
                    phaseE_posadd(1)
                    phaseCa_fin(1, a1_all)
                    phaseDy(1)
                    phaseDu(1, a1_all)      # cc1 in flight...
                    phaseE_udma(0, a0_all)
                    phaseE_chain(0, layer, a0_all)
                    phaseE_udma(1, a1_all)
                    if not last:
                        phaseB_px(0)
                        phaseE_chain(1, layer, a1_all)
                        phaseB_rope(0)
                        phaseG(0)
                        phaseB_px(1)
                        phaseB_rope(1)
                        phaseG(1)
                    else:
                        if DO_READOUT:
                            readout_half(0)
                        phaseE_chain(1, layer, a1_all)
                        if DO_READOUT:
                            readout_half(1)

    nc.compile()
    return nc


_NC_CACHE = None


def _get_nc():
    global _NC_CACHE
    if _NC_CACHE is None:
        nc = bacc.Bacc("TRN2", target_bir_lowering=False, debug=False, num_devices=8)
        _NC_CACHE = build(nc)
    return _NC_CACHE


def _rope_tables():
    # match the jax reference: float32 angle computation, then bf16 cast
    import ml_dtypes
    inv_freq = (1.0 / (10000.0 ** (np.arange(0, K, 2, dtype=np.float32)
                                   / np.float32(K)))).astype(np.float32)
    t = np.arange(T, dtype=np.float32)
    freqs = (t[:, None] * inv_freq[None, :]).astype(np.float32)  # [T, K/2]
    cos = np.cos(freqs).astype(np.float32)
    sin = np.sin(freqs).astype(np.float32)
    # [K/2, T] -> [4, 128, 2, 1024] -> [8, 128, 1024] with index i*2+th
    def pack(a):
        aT = np.ascontiguousarray(a.T).reshape(4, 128, 2, TH)
        return np.ascontiguousarray(
            aT.transpose(0, 2, 1, 3).reshape(8, 128, TH)).astype(ml_dtypes.bfloat16)
    return pack(cos), pack(sin)


def kernel(input_, emb, pos, Dx, Dy, E, readout):
    import ml_dtypes
    BF = ml_dtypes.bfloat16
    input_ = np.asarray(input_)
    emb = np.ascontiguousarray(np.asarray(emb, dtype=np.float32))
    pos = np.ascontiguousarray(np.asarray(pos, dtype=np.float32))
    Dx = np.asarray(Dx, dtype=np.float32)
    Dy = np.asarray(Dy, dtype=np.float32)
    E = np.asarray(E, dtype=np.float32)
    readout = np.asarray(readout, dtype=np.float32)

    nc = _get_nc()
    cosb, sinb = _rope_tables()
    ro_bf = readout.astype(BF)

    in_maps = []
    for c in range(8):
        b, h = divmod(c, 4)
        in_maps.append({
            "tok": np.ascontiguousarray(input_[b].astype(np.int32)),
            "emb": emb,
            "posb": np.ascontiguousarray(pos.astype(BF)),
            "dxb": np.ascontiguousarray(Dx[h].astype(BF)),
            "dyb": np.ascontiguousarray(Dy[h].astype(BF)),
            "eb": np.ascontiguousarray(E[h * K:(h + 1) * K].astype(BF)),
            "rob": np.ascontiguousarray(ro_bf[:, h * VS:(h + 1) * VS]),
            "cosb": cosb,
            "sinb": sinb,
        })
    trace = os.environ.get("KRN_TRACE", "0") == "1"
    res = run_bass_kernel_spmd(nc, in_maps, list(range(8)), trace=trace)
    out = np.empty((B, T, V), dtype=np.float32)
    for c in range(8):
        b, h = divmod(c, 4)
        out[b, :, h * VS:(h + 1) * VS] = res.results[c]["logitsT"].astype(np.float32).T
    kernel._last_results = res
    return out


# revision 27
# speedup vs baseline: 1.0136x; 1.0136x over previous
"""Trainium2 Bass kernel for nn_BDH_4406636445711 (dense transformer).

Sharding: 8 cores = data-parallel over B(2) x tensor-parallel over H(4).
Core c handles (b = c//4, h = c%4): its head's Dx/Dy slices, E rows, and a
V/4 shard of the readout. Per layer the y@E partial is AllReduced (bf16)
within each b-group of 4 cores. The host stitches the 8 per-core [VS, T]
logit shards (bf16 on device, cast to fp32 host-side) into [B, T, V].

Key algebraic optimization vs the naive graph: scores = q @ q^T is only
ever used for a = scores @ v, so we compute a = q @ (q^T v) instead --
G = q^T v is [K, D]; ~5x fewer PE cycles than materializing [T, T] scores.

Layouts: v lives token-major ("td": [128 tok part, 16 chunk, 256 d]) so
every LayerNorm is a free-dim reduction. x/q live kT; q is additionally
transposed to tk (via the DMA crossbar, not the PE) for the G matmul.
All matmul operands are bf16 (full PE rate); accumulation and LN stats
stay fp32.

Scheduling structure (v3):
- Each token half runs attention->Dy->Du back to back, so its AllReduce
  launches a half-phase earlier and rides under the other half's matmuls.
- LN statistics use one-pass bn_stats/bn_aggr; apply + residual-add +
  the vp transpose are chunk-pipelined across ACT/DVE/PE so the
  post-collective chain is short and px can start after 4 chunks.
- q's kT->tk transpose rides the DMA crossbar via a DRAM round-trip,
  with dispatches split across the SP and ACT hwdge queues.
- The readout streams weights in 512-column chunks on a 4-deep
  [128,512] PSUM pool, alternating DVE/ACT for the PSUM->SBUF casts.
"""

import os
import sys

sys.path.insert(0, "/opt/trn_rl_repo")

import numpy as np

import concourse.bass as bass
import concourse.tile as tile
from concourse import bacc, mybir
from concourse.bass_utils import run_bass_kernel_spmd
from concourse.masks import make_identity
from concourse import library_config

F32 = mybir.dt.float32
BF16 = mybir.dt.bfloat16
I32 = mybir.dt.int32
AF = mybir.ActivationFunctionType
OP = mybir.AluOpType
AX = mybir.AxisListType

B, T, H, D, K, V, L = 2, 2048, 4, 256, 1024, 32000, 6
VS = V // 4          # vocab shard per core within a b-group
EPS = 1e-5
NT = T // 128        # 16 token chunks
NKC = K // 128       # 8 k chunks
ND = D // 128        # 2 d chunks
TH = T // 2          # 1024

N_LAYERS = int(os.environ.get("KRN_LAYERS", str(L)))
DO_READOUT = os.environ.get("KRN_READOUT", "1") == "1"


def build(nc):
    # ---- DRAM parameters (per core) ----
    tok_d = nc.dram_tensor("tok", [T], I32, kind="ExternalInput")
    emb_d = nc.dram_tensor("emb", [V, D], F32, kind="ExternalInput")
    pos_d = nc.dram_tensor("posb", [T, D], BF16, kind="ExternalInput")
    dx_d = nc.dram_tensor("dxb", [D, K], BF16, kind="ExternalInput")
    dy_d = nc.dram_tensor("dyb", [D, K], BF16, kind="ExternalInput")
    e_d = nc.dram_tensor("eb", [K, D], BF16, kind="ExternalInput")
    ro_d = nc.dram_tensor("rob", [D, VS], BF16, kind="ExternalInput")
    cos_d = nc.dram_tensor("cosb", [8, 128, TH], BF16, kind="ExternalInput")
    sin_d = nc.dram_tensor("sinb", [8, 128, TH], BF16, kind="ExternalInput")
    out_d = nc.dram_tensor("logitsT", [VS, T], BF16, kind="ExternalOutput")

    groups = [[0, 1, 2, 3], [4, 5, 6, 7]]

    with tile.TileContext(nc) as tc:
        with (
            nc.allow_low_precision(reason="bf16 matmul path is intentional"),
            tc.tile_pool(name="persist", bufs=1) as pp,
            tc.tile_pool(name="w4", bufs=4) as w4p,     # [128,1024] bf16 rope
            tc.tile_pool(name="stats", bufs=24) as stp, # [128,8] f32
            tc.tile_pool(name="bst", bufs=12) as bsp,   # [128,8,6]/[128,8,2] f32
            tc.tile_pool(name="rop", bufs=2) as rop,    # readout weights
            tc.tile_pool(name="lop", bufs=2) as lop,    # logit staging
            tc.tile_pool(name="pb", bufs=4, space="PSUM") as pbp,   # [128,512] f32
            tc.tile_pool(name="pa", bufs=2, space="PSUM") as pap,   # [128,256] f32
            tc.tile_pool(name="pt", bufs=2, space="PSUM") as ptp,   # [128,1024] bf16
            tc.tile_pool(name="dram", bufs=1, space="DRAM") as dpool,
        ):
            _ctr = [0]

            def _nm(p):
                _ctr[0] += 1
                return f"{p}{_ctr[0]}"

            # ---- constants ----
            ident_f = pp.tile([128, 128], F32)
            make_identity(nc, ident_f[:])
            ident_bf = pp.tile([128, 128], BF16)
            nc.vector.tensor_copy(ident_bf[:], ident_f[:])
            eps_p = pp.tile([128, 1], F32)
            nc.vector.memset(eps_p[:], EPS)
            nc.gpsimd.load_library(library_config.attn)

            # ---- persistent tensors ----
            v_td = pp.tile([128, NT, D], F32)          # ln(w), token-major (no pos)
            vp_bf = pp.tile([128, NT, D], BF16)        # v + pos, bf16
            vpT = pp.tile([128, ND, T], BF16)          # (v+pos) transposed
            qT = pp.tile([128, NKC, T], BF16)          # q k-major; reused as yt
            qtk = pp.tile([128, NT, K], BF16)          # q token-major
            x_bf = pp.tile([128, NKC, T], BF16)        # relu(v@Dx), k-major
            g0_sb = pp.tile([128, NKC, D], BF16)       # G partial (th0 tokens)
            g_bf = pp.tile([128, NKC, D], BF16)        # G = q^T (v+pos), full
            lnA_dT = pp.tile([128, ND, T], BF16)       # ln(a) d-major
            pos_sb = pp.tile([128, NT, D], BF16)
            dx_sb = pp.tile([128, ND, K], BF16)
            dy_sb = pp.tile([128, ND, K], BF16)
            e_sb = pp.tile([128, NKC, D], BF16)
            # half-batch scratch ([128, 8, 256] = one token half). Multi-purpose;
            # phase ordering keeps uses disjoint (WAR tracked by the framework).
            a0_all = pp.tile([128, 8, D], BF16)        # th0: a / u-stage / u
            a1_all = pp.tile([128, 8, D], BF16)        # th1: a / u-stage / u
            uln_all = pp.tile([128, 8, D], F32)        # ln(u) / embed gathers

            nc.sync.dma_start(pos_sb[:], pos_d.ap().rearrange("(j p) d -> p j d", p=128))
            nc.sync.dma_start(dx_sb[:], dx_d.ap().rearrange("(c p) k -> p c k", p=128))
            nc.sync.dma_start(dy_sb[:], dy_d.ap().rearrange("(c p) k -> p c k", p=128))
            nc.sync.dma_start(e_sb[:], e_d.ap().rearrange("(c p) d -> p c d", p=128))

            # ---- internal DRAM ----
            cc_in = [dpool.tile([8, 128, D], BF16, tag=f"cci{i}", name=f"cci{i}")
                     for i in range(2)]
            cc_out = [dpool.tile([8, 128, D], BF16, tag=f"cco{i}", name=f"cco{i}")
                      for i in range(2)]
            q_dr = dpool.tile([K, T], BF16, tag="qdr", name="qdr")

            def ln_tail(mv):
                """(mean, var) [128,8,2] -> (rstd, nmr) [128,8]."""
                sd = stp.tile([128, 8], F32, tag="st", name=_nm("st_"))
                nc.scalar.activation(sd[:], mv[:, :, 1], AF.Sqrt, bias=eps_p[:])
                rstd = stp.tile([128, 8], F32, tag="st", name=_nm("st_"))
                nc.vector.reciprocal(rstd[:], sd[:])
                nmr = stp.tile([128, 8], F32, tag="st", name=_nm("st_"))
                nc.vector.scalar_tensor_tensor(
                    out=nmr[:], in0=mv[:, :, 0], scalar=-1.0, in1=rstd[:],
                    op0=OP.mult, op1=OP.mult)
                return rstd, nmr

            def ln_stats(src3d):
                """One-pass LN stats of [128, 8, 256] -> (rstd, nmr) [128,8]."""
                bst = bsp.tile([128, 8, 6], F32, tag="bst6", name=_nm("bs_"))
                for j in range(8):
                    nc.vector.bn_stats(bst[:, j, :], src3d[:, j, :])
                mv = bsp.tile([128, 8, 2], F32, tag="bst2", name=_nm("bs_"))
                for j in range(8):
                    nc.vector.bn_aggr(mv[:, j, :], bst[:, j, :])
                return ln_tail(mv)

            def apply_half(dst_tile, dst0, src_tile, src0, rstd, nmr):
                for j8 in range(8):
                    nc.scalar.activation(dst_tile[:, dst0 + j8],
                                         src_tile[:, src0 + j8], AF.Identity,
                                         bias=nmr[:, j8:j8 + 1],
                                         scale=rstd[:, j8:j8 + 1])

            def transpose_half(src_tile, sl, dst, c0):
                """Transpose 8 [128, 256] td chunks into dst[:, dc, c0:c0+1024]."""
                tpa = ptp.tile([128, TH], BF16, tag="pt", name=_nm("pt_"))
                tpb = ptp.tile([128, TH], BF16, tag="pt", name=_nm("pt_"))
                for j8 in range(8):
                    nc.tensor.transpose(tpa[:, j8 * 128:(j8 + 1) * 128],
                                        src_tile[:, sl + j8, 0:128], ident_bf[:])
                    nc.tensor.transpose(tpb[:, j8 * 128:(j8 + 1) * 128],
                                        src_tile[:, sl + j8, 128:256], ident_bf[:])
                nc.scalar.copy(dst[:, 0, c0:c0 + TH], tpa[:])
                nc.scalar.copy(dst[:, 1, c0:c0 + TH], tpb[:])

            def phaseB_px(th):
                """x[:, th cols] = relu(vp @ Dx)."""
                c0 = th * TH

                def px_one(i):
                    for ns in range(2):
                        px = pbp.tile([128, 512], F32, tag="pb", name=_nm("pb_"))
                        for dc in range(ND):
                            nc.tensor.matmul(
                                px[:],
                                dx_sb[:, dc, i * 128:(i + 1) * 128],
                                vpT[:, dc, c0 + ns * 512:c0 + (ns + 1) * 512],
                                start=(dc == 0), stop=(dc == ND - 1))
                        nc.scalar.activation(
                            x_bf[:, i, c0 + ns * 512:c0 + (ns + 1) * 512],
                            px[:], AF.Relu)

                for i in range(NKC):
                    px_one(i)

            def phaseB_rope(th):
                """RoPE -> q; q -> DRAM -> qtk (one batched xbar transpose)."""
                c0 = th * TH

                def rope_one(i):
                    """q_i on DVE, q_{i+4} mostly on Pool (parallel chains)."""
                    cos_t = w4p.tile([128, TH], BF16, tag="w4", name=_nm("w4_"))
                    nc.sync.dma_start(cos_t[:], cos_d.ap()[i * 2 + th])
                    sin_t = w4p.tile([128, TH], BF16, tag="w4", name=_nm("w4_"))
                    nc.sync.dma_start(sin_t[:], sin_d.ap()[i * 2 + th])
                    xi = x_bf[:, i, c0:c0 + TH]
                    xj = x_bf[:, i + 4, c0:c0 + TH]
                    ma = w4p.tile([128, TH], BF16, tag="w4", name=_nm("w4_"))
                    nc.vector.tensor_mul(ma[:], xi, cos_t[:])
                    mb = w4p.tile([128, TH], BF16, tag="w4", name=_nm("w4_"))
                    nc.vector.tensor_mul(mb[:], xj, sin_t[:])
                    nc.vector.tensor_sub(qT[:, i, c0:c0 + TH], ma[:], mb[:])
                    nc.vector.tensor_mul(ma[:], xj, cos_t[:])
                    nc.vector.tensor_mul(mb[:], xi, sin_t[:])
                    nc.vector.tensor_add(qT[:, i + 4, c0:c0 + TH], ma[:], mb[:])
                    nc.sync.dma_start(q_dr[i * 128:(i + 1) * 128, c0:c0 + TH],
                                      qT[:, i, c0:c0 + TH])
                    nc.sync.dma_start(
                        q_dr[(i + 4) * 128:(i + 5) * 128, c0:c0 + TH],
                        qT[:, i + 4, c0:c0 + TH])

                rope_one(0); rope_one(1); rope_one(2); rope_one(3)
                # NOTE: xbar transposes dispatched from the ACT hwdge queue
                # return corrupted data on HW -- keep them on SP. One batched
                # dispatch: out[t, j, k] = q_dr[k, c0 + j*128 + t].
                nc.sync.dma_start_transpose(
                    qtk[:, th * 8:th * 8 + 8, :], q_dr[:, c0:c0 + TH])

            def phaseB(th):
                phaseB_px(th)
                phaseB_rope(th)

            def phaseG(half):
                """G half-accumulation over token chunks; half 1 finalizes g_bf."""
                for kc in range(NKC):
                    pg = pap.tile([128, D], F32, tag="pa", name=_nm("pa_"))
                    for j in range(half * 8, half * 8 + 8):
                        nc.tensor.matmul(pg[:], qtk[:, j, kc * 128:(kc + 1) * 128],
                                         vp_bf[:, j],
                                         start=(j == half * 8), stop=(j == half * 8 + 7))
                    if half == 0:
                        nc.scalar.copy(g0_sb[:, kc], pg[:])
                    else:
                        nc.vector.tensor_add(g_bf[:, kc], g0_sb[:, kc], pg[:])

            def phaseCa_mm(half, dst_tile):
                """a = q G matmuls for a token half; PSUM -> SBUF bf16."""
                h0 = half * 8
                for j8 in range(8):
                    j = h0 + j8
                    paa = pap.tile([128, D], F32, tag="pa", name=_nm("pa_"))
                    for kc in range(NKC):
                        nc.tensor.matmul(paa[:], qT[:, kc, j * 128:(j + 1) * 128],
                                         g_bf[:, kc],
                                         start=(kc == 0), stop=(kc == NKC - 1))
                    nc.scalar.copy(dst_tile[:, j8], paa[:])

            def phaseCa_fin(half, src_tile):
                """Batched LN of the a half (in place) -> lnA_dT, chunk-piped."""
                c0 = half * TH
                rstd, nmr = ln_stats(src_tile[:])
                tpa = ptp.tile([128, TH], BF16, tag="pt", name=_nm("pt_"))
                tpb = ptp.tile([128, TH], BF16, tag="pt", name=_nm("pt_"))
                for j in range(8):
                    nc.scalar.activation(src_tile[:, j], src_tile[:, j],
                                         AF.Identity, bias=nmr[:, j:j + 1],
                                         scale=rstd[:, j:j + 1])
                    nc.tensor.transpose(tpa[:, j * 128:(j + 1) * 128],
                                        src_tile[:, j, 0:128], ident_bf[:])
                    nc.tensor.transpose(tpb[:, j * 128:(j + 1) * 128],
                                        src_tile[:, j, 128:256], ident_bf[:])
                    if j == 3:
                        nc.scalar.copy(lnA_dT[:, 0, c0:c0 + 512], tpa[:, 0:512])
                        nc.scalar.copy(lnA_dT[:, 1, c0:c0 + 512], tpb[:, 0:512])
                nc.scalar.copy(lnA_dT[:, 0, c0 + 512:c0 + TH], tpa[:, 512:TH])
                nc.scalar.copy(lnA_dT[:, 1, c0 + 512:c0 + TH], tpb[:, 512:TH])

            def phaseDy(th):
                """y = relu(lnA@Dy)*x into yt (aliases q's buffer)."""
                c0 = th * TH
                yt = qT
                for i in range(NKC):
                    for ns in range(2):
                        py = pbp.tile([128, 512], F32, tag="pb", name=_nm("pb_"))
                        for dc in range(ND):
                            nc.tensor.matmul(
                                py[:],
                                dy_sb[:, dc, i * 128:(i + 1) * 128],
                                lnA_dT[:, dc, c0 + ns * 512:c0 + (ns + 1) * 512],
                                start=(dc == 0), stop=(dc == ND - 1))
                        nc.vector.scalar_tensor_tensor(
                            out=yt[:, i, c0 + ns * 512:c0 + (ns + 1) * 512],
                            in0=py[:], scalar=0.0,
                            in1=x_bf[:, i, c0 + ns * 512:c0 + (ns + 1) * 512],
                            op0=OP.max, op1=OP.mult)

            def phaseDu(th, stage):
                """u = y@E (token-major); stage bf16 and AllReduce."""
                yt = qT
                for j8 in range(8):
                    j = th * 8 + j8
                    pu = pap.tile([128, D], F32, tag="pa", name=_nm("pa_"))
                    for i in range(NKC):
                        nc.tensor.matmul(pu[:], yt[:, i, j * 128:(j + 1) * 128],
                                         e_sb[:, i],
                                         start=(i == 0), stop=(i == NKC - 1))
                    nc.scalar.copy(stage[:, j8], pu[:])
                    if j8 == 3:
                        nc.sync.dma_start(
                            cc_in[th][0:4].rearrange("j p d -> p j d"),
                            stage[:, 0:4])
                nc.sync.dma_start(
                    cc_in[th][4:8].rearrange("j p d -> p j d"),
                    stage[:, 4:8])
                nc.gpsimd.collective_compute(
                    "AllReduce", OP.add, replica_groups=groups,
                    ins=[cc_in[th][:].opt()], outs=[cc_out[th][:].opt()])

            def phaseE_posadd(th):
                sl = slice(th * 8, th * 8 + 8)
                nc.vector.tensor_add(v_td[:, sl], v_td[:, sl], pos_sb[:, sl])

            def phaseE_udma(th, ubuf):
                """Bring the AllReduced u back in two chunks."""
                for hh in range(2):
                    nc.sync.dma_start(
                        ubuf[:, hh * 4:(hh + 1) * 4],
                        cc_out[th][hh * 4:(hh + 1) * 4].rearrange("j p d -> p j d"))

            def phaseE_chain_mono(th, layer, ubuf):
                """v2-style monolithic E chain for bisection."""
                last = layer == N_LAYERS - 1
                h0 = th * 8
                sl = slice(h0, h0 + 8)
                rstd_u, nmr_u = ln_stats(ubuf[:])
                apply_half(uln_all, 0, ubuf, 0, rstd_u, nmr_u)
                nc.vector.tensor_add(v_td[:, sl], v_td[:, sl], uln_all[:])
                rstd_w, nmr_w = ln_stats(v_td[:, sl])
                apply_half(v_td, h0, v_td, h0, rstd_w, nmr_w)
                if not last:
                    nc.vector.tensor_add(vp_bf[:, sl], v_td[:, sl], pos_sb[:, sl])
                else:
                    nc.scalar.copy(vp_bf[:, sl], v_td[:, sl])
                transpose_half(vp_bf, h0, vpT, th * TH)

            def phaseE_chain_chunked(th, layer, ubuf):
                """w = (v+pos)+ln(u); v = ln(w); vp = v+pos'; vpT. Chunk-piped
                across ACT/DVE/PE so px can start after 4 chunks."""
                last = layer == N_LAYERS - 1
                h0 = th * 8
                c0 = th * TH
                rstd_u, nmr_u = ln_stats(ubuf[:])
                bstw = bsp.tile([128, 8, 6], F32, tag="bst6", name=_nm("bs_"))
                for j in range(8):
                    nc.scalar.activation(uln_all[:, j], ubuf[:, j], AF.Identity,
                                         bias=nmr_u[:, j:j + 1],
                                         scale=rstd_u[:, j:j + 1])
                    nc.vector.tensor_add(v_td[:, h0 + j], v_td[:, h0 + j],
                                         uln_all[:, j])
                    nc.vector.bn_stats(bstw[:, j, :], v_td[:, h0 + j])
                mvw = bsp.tile([128, 8, 2], F32, tag="bst2", name=_nm("bs_"))
                for j in range(8):
                    nc.vector.bn_aggr(mvw[:, j, :], bstw[:, j, :])
                rstd_w, nmr_w = ln_tail(mvw)
                tpa = ptp.tile([128, TH], BF16, tag="pt", name=_nm("pt_"))
                tpb = ptp.tile([128, TH], BF16, tag="pt", name=_nm("pt_"))
                for j in range(8):
                    nc.scalar.activation(v_td[:, h0 + j], v_td[:, h0 + j],
                                         AF.Identity, bias=nmr_w[:, j:j + 1],
                                         scale=rstd_w[:, j:j + 1])
                    if not last:
                        nc.vector.tensor_add(vp_bf[:, h0 + j], v_td[:, h0 + j],
                                             pos_sb[:, h0 + j])
                    else:
                        nc.scalar.copy(vp_bf[:, h0 + j], v_td[:, h0 + j])
                    nc.tensor.transpose(tpa[:, j * 128:(j + 1) * 128],
                                        vp_bf[:, h0 + j, 0:128], ident_bf[:])
                    nc.tensor.transpose(tpb[:, j * 128:(j + 1) * 128],
                                        vp_bf[:, h0 + j, 128:256], ident_bf[:])
                    if j == 3:
                        nc.scalar.copy(vpT[:, 0, c0:c0 + 512], tpa[:, 0:512])
                        nc.scalar.copy(vpT[:, 1, c0:c0 + 512], tpb[:, 0:512])
                nc.scalar.copy(vpT[:, 0, c0 + 512:c0 + TH], tpa[:, 512:TH])
                nc.scalar.copy(vpT[:, 1, c0 + 512:c0 + TH], tpb[:, 512:TH])

            phaseE_chain = (phaseE_chain_chunked
                            if os.environ.get("KRN_CHAIN", "1") == "1"
                            else phaseE_chain_mono)

            # vocab block structure for the readout: 63 blocks of 128 (last 64),
            # weight chunks of 4 blocks (512 cols), stores in vblock pairs.
            VBW = [128] * 62 + [64]
            VCHUNKS = []
            vb = 0
            while vb < 63:
                VCHUNKS.append(list(range(vb, min(vb + 4, 63))))
                vb += 4
            _cp = [0]

            def readout_half(th):
                """logitsT[:, th cols] = (v @ readout)^T for the token half."""
                for chunk in VCHUNKS:
                    off = chunk[0] * 128
                    w = sum(VBW[i] for i in chunk)
                    rot = rop.tile([128, ND, 512], BF16, tag="ro", name=_nm("ro_"))
                    for dc in range(ND):
                        nc.sync.dma_start(
                            rot[:, dc, :w],
                            ro_d.ap()[dc * 128:(dc + 1) * 128, off:off + w])
                    pairs = [chunk[i:i + 2] for i in range(0, len(chunk), 2)]
                    for pair in pairs:
                        lo = lop.tile([128, len(pair), TH], BF16, tag="lo",
                                      name=_nm("lo_"))
                        for mi, vbi in enumerate(pair):
                            m = VBW[vbi]
                            loc = vbi * 128 - off
                            for ns in range(2):
                                pl = pbp.tile([128, 512], F32, tag="pb",
                                              name=_nm("pb_"))
                                for dc in range(ND):
                                    nc.tensor.matmul(
                                        pl[:m],
                                        rot[:, dc, loc:loc + m],
                                        vpT[:, dc, th * TH + ns * 512:
                                            th * TH + (ns + 1) * 512],
                                        start=(dc == 0), stop=(dc == ND - 1))
                                _cp[0] += 1
                                dst = lo[:m, mi, ns * 512:(ns + 1) * 512]
                                if _cp[0] % 2 == 0:
                                    nc.vector.tensor_copy(dst, pl[:m])
                                else:
                                    nc.scalar.copy(dst, pl[:m])
                        r0 = pair[0] * 128
                        rows = sum(VBW[i] for i in pair)
                        if len(pair) == 2 and rows == 256:
                            nc.sync.dma_start(
                                out_d.ap()[r0:r0 + 256, th * TH:(th + 1) * TH]
                                .rearrange("(vb p) t -> p vb t", p=128),
                                lo[:])
                        else:
                            nc.sync.dma_start(
                                out_d.ap()[r0:r0 + rows, th * TH:(th + 1) * TH],
                                lo[:rows, 0])

            # ======================= prologue: gather + LN =======================
            idx = pp.tile([128, NT], I32)
            nc.sync.dma_start(idx[:], tok_d.ap().rearrange("(n p) -> p n", p=128))

            def embed_gather(th):
                for j8 in range(8):
                    nc.gpsimd.indirect_dma_start(
                        out=uln_all[:, j8], out_offset=None, in_=emb_d.ap(),
                        in_offset=bass.IndirectOffsetOnAxis(
                            ap=idx[:, th * 8 + j8:th * 8 + j8 + 1], axis=0),
                    )

            def embed_half(th):
                h0 = th * 8
                sl = slice(h0, h0 + 8)
                rstd, nmr = ln_stats(uln_all[:])
                apply_half(v_td, h0, uln_all, 0, rstd, nmr)
                nc.vector.tensor_add(vp_bf[:, sl], v_td[:, sl], pos_sb[:, sl])
                transpose_half(vp_bf, h0, vpT, th * TH)

            embed_gather(0)
            embed_half(0)
            embed_gather(1)   # WAR on uln_all: starts once half0's apply read it
            phaseB(0)
            embed_half(1)
            phaseB(1)
            phaseG(0)
            phaseG(1)

            # ================================ layers ================================
            for layer in range(N_LAYERS):
                last = layer == N_LAYERS - 1
                with nc.named_scope(f"L{layer}"):
                    phaseCa_mm(0, a0_all)
                    phaseE_posadd(0)
                    phaseCa_fin(0, a0_all)
                    phaseDy(0)
                    phaseDu(0, a0_all)      # cc0 in flight...
                    phaseCa_mm(1, a1_all)
                    phaseE_posadd(1)
                    phaseCa_fin(1, a1_all)
                    phaseDy(1)
                    phaseDu(1, a1_all)      # cc1 in flight...
                    phaseE_udma(0, a0_all)
                    phaseE_chain(0, layer, a0_all)
                    phaseE_udma(1, a1_all)
                    if not last:
                        phaseB_px(0)
                        phaseE_chain(1, layer, a1_all)
                        phaseB_rope(0)
                        phaseG(0)
                        phaseB_px(1)
                        phaseB_rope(1)
                        phaseG(1)
                    else:
                        if DO_READOUT:
                            readout_half(0)
                        phaseE_chain(1, layer, a1_all)
                        if DO_READOUT:
                            readout_half(1)

    nc.compile()
    return nc


_NC_CACHE = None


def _get_nc():
    global _NC_CACHE
    if _NC_CACHE is None:
        nc = bacc.Bacc("TRN2", target_bir_lowering=False, debug=False, num_devices=8)
        _NC_CACHE = build(nc)
    return _NC_CACHE


def _rope_tables():
    # match the jax reference: float32 angle computation, then bf16 cast
    import ml_dtypes
    inv_freq = (1.0 / (10000.0 ** (np.arange(0, K, 2, dtype=np.float32)
                                   / np.float32(K)))).astype(np.float32)
    t = np.arange(T, dtype=np.float32)
    freqs = (t[:, None] * inv_freq[None, :]).astype(np.float32)  # [T, K/2]
    cos = np.cos(freqs).astype(np.float32)
    sin = np.sin(freqs).astype(np.float32)
    # [K/2, T] -> [4, 128, 2, 1024] -> [8, 128, 1024] with index i*2+th
    def pack(a):
        aT = np.ascontiguousarray(a.T).reshape(4, 128, 2, TH)
        return np.ascontiguousarray(
            aT.transpose(0, 2, 1, 3).reshape(8, 128, TH)).astype(ml_dtypes.bfloat16)
    return pack(cos), pack(sin)


def kernel(input_, emb, pos, Dx, Dy, E, readout):
    import ml_dtypes
    BF = ml_dtypes.bfloat16
    input_ = np.asarray(input_)
    emb = np.ascontiguousarray(np.asarray(emb, dtype=np.float32))
    pos = np.ascontiguousarray(np.asarray(pos, dtype=np.float32))
    Dx = np.asarray(Dx, dtype=np.float32)
    Dy = np.asarray(Dy, dtype=np.float32)
    E = np.asarray(E, dtype=np.float32)
    readout = np.asarray(readout, dtype=np.float32)

    nc = _get_nc()
    cosb, sinb = _rope_tables()
    ro_bf = readout.astype(BF)

    in_maps = []
    for c in range(8):
        b, h = divmod(c, 4)
        in_maps.append({
            "tok": np.ascontiguousarray(input_[b].astype(np.int32)),
            "emb": emb,
            "posb": np.ascontiguousarray(pos.astype(BF)),
            "dxb": np.ascontiguousarray(Dx[h].astype(BF)),
            "dyb": np.ascontiguousarray(Dy[h].astype(BF)),
            "eb": np.ascontiguousarray(E[h * K:(h + 1) * K].astype(BF)),
            "rob": np.ascontiguousarray(ro_bf[:, h * VS:(h + 1) * VS]),
            "cosb": cosb,
            "sinb": sinb,
        })
    trace = os.environ.get("KRN_TRACE", "0") == "1"
    res = run_bass_kernel_spmd(nc, in_maps, list(range(8)), trace=trace)
    out = np.empty((B, T, V), dtype=np.float32)
    for c in range(8):
        b, h = divmod(c, 4)
        out[b, :, h * VS:(h + 1) * VS] = res.results[c]["logitsT"].astype(np.float32).T
    kernel._last_results = res
    return out
